# revision 7
# baseline (speedup 1.0000x reference)
"""Trainium2 Bass kernel for a GPT-style transformer block.

B=4, T=2048, C=1024, H=16 heads (hd=64), D_FF=4096, fp32 I/O,
pre-LN, non-causal attention, tanh-approx GELU.

Sharding: 8 cores = 4 batch elements x 2 token-halves. Each core
computes attention K/V for its full batch element (dup of the K/V
projection for the other half -- avoids all collectives) and Q/MLP for
its own 1024 tokens. Host reorders tokens so each core's own tokens are
always rows 0..1023 -> identical NEFF on all 8 cores.

v2: bf16 activations/weights through attention (fast transposes, light
DMA), K/Q projection fused into the per-head-pair attention loop so exp
on the Activation engine overlaps PE work, stationary-reuse loop orders
(halved LDWEIGHTS), fp8e4m3 DoubleRow FFN (2 contraction rows per
partition), per-phase named scopes.
"""

import numpy as np
from contextlib import ExitStack

import concourse.bass as bass
import concourse.bacc as bacc
import concourse.mybir as mybir
from concourse import tile
from concourse.bass_utils import run_bass_kernel_spmd
from concourse.masks import make_identity

F32 = mybir.dt.float32
BF16 = mybir.dt.bfloat16
FP8 = mybir.dt.float8e4
AF = mybir.ActivationFunctionType
ALU = mybir.AluOpType
DR = mybir.MatmulPerfMode.DoubleRow

P = 128
T = 2048      # tokens per batch element (per core: kv tokens)
TO = 1024     # own tokens per core
C = 1024
H = 16
HD = 64
FF = 4096
NT = T // P   # 16 token tiles (kv)
NTO = TO // P  # 8 own token tiles
NC = C // P   # 8 channel tiles
NF = FF // P  # 32 ff tiles
EPS = 1e-5

USE_FP8_FFN = True
WFC_SCALE = 8.0
WPJ_SCALE = 16.0

_CACHE = {}
LAST_RESULT = None


def _ln_tile(nc, pools, src_ap, dstT, tslice, ident16, epsc):
    """LayerNorm one [128, C] token tile (gains folded into weights on
    host), cast to bf16, transpose into dstT[:, :, tslice] (dstT dtype)."""
    pool, spool, pps = pools
    st = spool.tile([P, 2, 6], F32, name="ln_st")
    for g in range(2):
        nc.vector.bn_stats(st[:, g], src_ap[:, g * 512:(g + 1) * 512])
    ag = spool.tile([P, 2], F32, name="ln_ag")
    nc.vector.bn_aggr(ag[:], st[:])
    std = spool.tile([P, 1], F32, name="ln_std")
    nc.scalar.activation(std[:], ag[:, 1:2], AF.Sqrt, bias=epsc)
    rinv = spool.tile([P, 1], F32, name="ln_rinv")
    nc.vector.reciprocal(rinv[:], std[:])
    xh = pool.tile([P, C], BF16, name="ln_xh")
    nc.vector.tensor_scalar(
        xh[:], src_ap, ag[:, 0:1], rinv[:], ALU.subtract, ALU.mult)
    for c in range(NC):
        tp = pps.tile([P, P], BF16, name="ln_tp")
        nc.tensor.transpose(tp[:], xh[:, c * P:(c + 1) * P], ident16)
        dst = dstT[:, c, tslice]
        if c % 2 == 0:
            nc.scalar.copy(dst, tp[:])
        else:
            nc.vector.tensor_copy(dst, tp[:])


def _build():
    nc = bacc.Bacc(None, target_bir_lowering=False)
    wdt = FP8 if USE_FP8_FFN else BF16

    # ---- DRAM I/O ----
    x_d = nc.dram_tensor("x", (T, C), F32, kind="ExternalInput")
    wq_d = nc.dram_tensor("wq", (C, C), BF16, kind="ExternalInput")
    wk_d = nc.dram_tensor("wk", (C, C), BF16, kind="ExternalInput")
    wv_d = nc.dram_tensor("wv", (C, C), BF16, kind="ExternalInput")
    bqk_d = nc.dram_tensor("bqk", (2 * C,), F32, kind="ExternalInput")
    wap_d = nc.dram_tensor("wap", (C, C), BF16, kind="ExternalInput")
    wfc_d = nc.dram_tensor("wfc", (C, FF), wdt, kind="ExternalInput")
    bfc_d = nc.dram_tensor("bfc", (FF,), F32, kind="ExternalInput")
    wpj_d = nc.dram_tensor("wpj", (FF, C), BF16, kind="ExternalInput")
    out_d = nc.dram_tensor("out", (TO, C), F32, kind="ExternalOutput")

    with tile.TileContext(nc) as tc, ExitStack() as top:
        cpool = top.enter_context(tc.tile_pool(name="const", bufs=1))
        epsc = cpool.tile([P, 1], F32, name="epsc")
        nc.vector.memset(epsc[:], EPS)
        ident16 = cpool.tile([P, P], BF16, name="ident16")
        make_identity(nc, ident16)
        ones64 = cpool.tile([1, HD], BF16, name="ones64")
        nc.vector.memset(ones64[:], 1.0)
        bqk_sb = cpool.tile([P, 2 * NC], F32, name="bqk_sb")
        nc.sync.dma_start(
            bqk_sb[:], bqk_d[:].rearrange("(j p) -> p j", p=P))
        bfc_sb = cpool.tile([P, NF], F32, name="bfc_sb")
        nc.sync.dma_start(
            bfc_sb[:], bfc_d[:].rearrange("(j p) -> p j", p=P))

        esA = top.enter_context(ExitStack())   # xhT (left)
        esBC = top.enter_context(ExitStack())  # vsb (right)
        # yT lives from attention through phase D; allocate its pool below
        # const (before bigA) so left-side pool pops stay LIFO.
        yTp = top.enter_context(tc.tile_pool(name="yTp", bufs=1))

        # ============ Phase A: LN1 + transpose + V projection ============
        bigA = esA.enter_context(tc.tile_pool(name="bigA", bufs=1))
        xhT = bigA.tile([P, NC, T], BF16, name="xhT")  # 4 MB
        vsb = esBC.enter_context(
            tc.tile_pool(name="vsbp", bufs=1, side="right")).tile(
            [P, NT, H * (HD + 1)], BF16, name="vsb")
        with nc.named_scope("phA_ln_v"), ExitStack() as esAV:
            lnp = (esAV.enter_context(tc.tile_pool(name="ln_work", bufs=3)),
                   esAV.enter_context(tc.tile_pool(name="ln_stat", bufs=6)),
                   esAV.enter_context(
                       tc.tile_pool(name="ln_ps", bufs=2, space="PSUM")))
            xpool = esAV.enter_context(tc.tile_pool(name="xinp", bufs=3))
            wvp = esAV.enter_context(tc.tile_pool(name="wvp", bufs=1))
            psV = esAV.enter_context(
                tc.tile_pool(name="psV", bufs=4, space="PSUM"))
            wv_sb = wvp.tile([P, NC, C], BF16, name="wv_sb")
            nc.sync.dma_start(
                wv_sb[:], wv_d[:].rearrange("(c p) o -> p c o", p=P))
            for i in range(NT):
                xt = xpool.tile([P, C], F32, name="ln_x")
                nc.sync.dma_start(xt[:], x_d[i * P:(i + 1) * P, :])
                _ln_tile(nc, lnp, xt[:], xhT,
                         slice(i * P, (i + 1) * P), ident16, epsc[:])
                # V proj: stationary = token tile of xhT, reused across vc
                psv = {vc: psV.tile([P, 512], F32, name="psv")
                       for vc in range(2)}
                for c in range(NC):
                    for vc in range(2):
                        nc.tensor.matmul(
                            psv[vc][:], xhT[:, c, i * P:(i + 1) * P],
                            wv_sb[:, c, vc * 512:(vc + 1) * 512],
                            start=(c == 0), stop=(c == NC - 1))
                dstv = vsb[:, i].rearrange("p (h e) -> p h e", e=HD + 1)
                for vc in range(2):
                    nc.vector.tensor_copy(
                        dstv[:, vc * 8:(vc + 1) * 8, :HD],
                        psv[vc][:].rearrange("p (h d) -> p h d", d=HD))
                nc.gpsimd.memset(dstv[:, :, HD:], 1.0)

        # ========== Phase B+C fused: K/Q proj + attention per j ==========
        yT = yTp.tile([P, NC, TO], BF16, name="yT")
        with nc.named_scope("phBC_attn"), ExitStack() as esC:
            wkp = esC.enter_context(tc.tile_pool(name="wkp", bufs=2))
            wqp = esC.enter_context(tc.tile_pool(name="wqp", bufs=2))
            kTp = esC.enter_context(tc.tile_pool(name="kTp", bufs=2,
                                                 side="right"))
            qTp = esC.enter_context(tc.tile_pool(name="qTp", bufs=2,
                                                 side="right"))
            psKQ = esC.enter_context(
                tc.tile_pool(name="psKQ", bufs=2, space="PSUM"))
            psS = esC.enter_context(
                tc.tile_pool(name="psS", bufs=2, space="PSUM"))
            psO = esC.enter_context(
                tc.tile_pool(name="psO", bufs=2, space="PSUM"))
            ppool = esC.enter_context(tc.tile_pool(name="pT", bufs=33))
            scrp = esC.enter_context(tc.tile_pool(name="scrp", bufs=2))
            dflp = esC.enter_context(tc.tile_pool(name="dflp", bufs=2))
            dinp = esC.enter_context(tc.tile_pool(name="dinp", bufs=2))
            wk_r = wk_d[:].rearrange("(c p) o -> p c o", p=P)
            wq_r = wq_d[:].rearrange("(c p) o -> p c o", p=P)
            for j in range(H // 2):
                wk_t = wkp.tile([P, NC, P], BF16, name="wk_t")
                nc.sync.dma_start(wk_t[:], wk_r[:, :, j * P:(j + 1) * P])
                wq_t = wqp.tile([P, NC, P], BF16, name="wq_t")
                nc.sync.dma_start(wq_t[:], wq_r[:, :, j * P:(j + 1) * P])
                kTj = kTp.tile([P, T], BF16, name="kTj")
                qTj = qTp.tile([P, TO], BF16, name="qTj")
                # K projection: stationary reused across 2 token chunks
                for tg in range(2):
                    pk = {t2: psKQ.tile([P, 512], F32, name="pskq")
                          for t2 in range(2)}
                    for c in range(NC):
                        for t2 in range(2):
                            toff = tg * 1024 + t2 * 512
                            nc.tensor.matmul(
                                pk[t2][:], wk_t[:, c],
                                xhT[:, c, toff:toff + 512],
                                start=(c == 0), stop=(c == NC - 1))
                    for t2 in range(2):
                        toff = tg * 1024 + t2 * 512
                        nc.vector.tensor_scalar_add(
                            kTj[:, toff:toff + 512], pk[t2][:],
                            bqk_sb[:, NC + j:NC + j + 1])
                # Q projection (own tokens only)
                pq = {t2: psKQ.tile([P, 512], F32, name="pskq")
                      for t2 in range(2)}
                for c in range(NC):
                    for t2 in range(2):
                        nc.tensor.matmul(
                            pq[t2][:], wq_t[:, c],
                            xhT[:, c, t2 * 512:(t2 + 1) * 512],
                            start=(c == 0), stop=(c == NC - 1))
                for t2 in range(2):
                    nc.vector.tensor_scalar_add(
                        qTj[:, t2 * 512:(t2 + 1) * 512], pq[t2][:],
                        bqk_sb[:, j:j + 1])
                # scores + exp; even/odd head pair on PE quadrants 0/64
                pT = {0: [None] * NT, 64: [None] * NT}
                for k in range(NT):
                    sps = {po: psS.tile([P, TO], F32, name="sps", tag="sps")
                           for po in (0, 64)}
                    for qc in range(2):
                        for po in (0, 64):
                            nc.tensor.matmul(
                                sps[po][:, qc * 512:(qc + 1) * 512],
                                kTj[po:po + HD, k * P:(k + 1) * P],
                                qTj[po:po + HD, qc * 512:(qc + 1) * 512],
                                start=True, stop=True)
                    for po in (0, 64):
                        pT[po][k] = ppool.tile([P, TO], BF16, name="pT_t")
                        nc.scalar.activation(
                            pT[po][k][:], sps[po][:], AF.Exp, scale=0.125)
                # wide PV: out = [V | 1]^T @ P -> [65, 512] = y^T rows +
                # denominator row; then divide via PE ones-broadcast of 1/d
                for po in (0, 64):
                    h = 2 * j + (po // HD)
                    for qg in range(2):
                        ops = psO.tile([P, 512], F32, name="ops")
                        for k in range(NT):
                            nc.tensor.matmul(
                                ops[:HD + 1, :],
                                vsb[:, k, h * (HD + 1):(h + 1) * (HD + 1)],
                                pT[po][k][:, qg * 512:(qg + 1) * 512],
                                start=(k == 0), stop=(k == NT - 1))
                        scr = scrp.tile([P, 512], BF16, name="scr")
                        nc.vector.tensor_copy(
                            scr[HD:HD + 1, :], ops[HD:HD + 1, :])
                        dfl = dflp.tile([1, 512], BF16, name="dfl")
                        nc.sync.dma_start(dfl[0:1, :], scr[HD:HD + 1, :])
                        dps = psO.tile([P, 512], F32, name="ops")
                        nc.tensor.matmul(
                            dps[:HD, :], ones64[0:1, :], dfl[0:1, :],
                            start=True, stop=True)
                        dinvT = dinp.tile([HD, 512], BF16, name="dinvT")
                        with nc.allow_low_precision(
                                reason="bf16 1/denom is plenty for softmax"):
                            nc.vector.reciprocal(dinvT[:], dps[:HD, :])
                        nc.vector.tensor_tensor(
                            yT[po:po + HD, j, qg * 512:(qg + 1) * 512],
                            ops[:HD, :], dinvT[:], ALU.mult)
        esA.close()   # free xhT
        esBC.close()  # free vsb

        # ========== Phase D: attn proj + residual + LN2 fused ==========
        x2 = top.enter_context(
            tc.tile_pool(name="x2p", bufs=1, side="right")).tile(
            [P, NTO, C], F32, name="x2")
        xh2T = top.enter_context(
            tc.tile_pool(name="bigE", bufs=1, side="right")).tile(
            [P, NC, TO], wdt, name="xh2T")
        wap_sb = top.enter_context(
            tc.tile_pool(name="wapp", bufs=1)).tile(
            [P, NC, C], BF16, name="wap_sb")
        nc.sync.dma_start(
            wap_sb[:], wap_d[:].rearrange("(c p) o -> p c o", p=P))
        # h2T (bf16, FFN1 out / FFN2 stationary) and wfc prefetch
        h2T = top.enter_context(
            tc.tile_pool(name="h2Tp", bufs=1, side="right")).tile(
            [P, NF, TO], BF16, name="h2T")
        wfc_sb = top.enter_context(
            tc.tile_pool(name="wfcp", bufs=1, side="right")).tile(
            [P, NC, FF], wdt, name="wfc_sb")
        nc.sync.dma_start(
            wfc_sb[:], wfc_d[:].rearrange("(c p) f -> p c f", p=P))
        with nc.named_scope("phD_proj_ln2"), ExitStack() as esD:
            xrp = esD.enter_context(tc.tile_pool(name="xrp", bufs=3))
            psD = esD.enter_context(
                tc.tile_pool(name="psD", bufs=4, space="PSUM"))
            ln2p = (esD.enter_context(tc.tile_pool(name="ln2_work", bufs=3)),
                    esD.enter_context(tc.tile_pool(name="ln2_stat", bufs=6)),
                    esD.enter_context(
                        tc.tile_pool(name="ln2_ps", bufs=2, space="PSUM")))
            for qt in range(NTO):
                xr = xrp.tile([P, C], F32, name="xr")
                nc.sync.dma_start(xr[:], x_d[qt * P:(qt + 1) * P, :])
                pd = {cc: psD.tile([P, 512], F32, name="psd")
                      for cc in range(2)}
                for c in range(NC):
                    for cc in range(2):
                        nc.tensor.matmul(
                            pd[cc][:], yT[:, c, qt * P:(qt + 1) * P],
                            wap_sb[:, c, cc * 512:(cc + 1) * 512],
                            start=(c == 0), stop=(c == NC - 1))
                for cc in range(2):
                    nc.vector.tensor_tensor(
                        x2[:, qt, cc * 512:(cc + 1) * 512], pd[cc][:],
                        xr[:, cc * 512:(cc + 1) * 512], ALU.add)
                _ln_tile(nc, ln2p, x2[:, qt], xh2T,
                         slice(qt * P, (qt + 1) * P), ident16, epsc[:])

        # ================= Phase F: FFN1 + gelu =================
        wpjp = top.enter_context(
            tc.tile_pool(name="wpjp", bufs=1, side="right"))
        wpj_r = wpj_d[:].rearrange("(f p) o -> p f o", p=P)
        wpj_t0 = wpjp.tile([P, NF, 512], BF16, name="wpj_t")
        nc.sync.dma_start(wpj_t0[:], wpj_r[:, :, 0:512])
        with nc.named_scope("phF_ffn1"), ExitStack() as esF:
            psF = esF.enter_context(
                tc.tile_pool(name="psF", bufs=4, space="PSUM"))
            for fj in range(NF):
                pf = {tch: psF.tile([P, 512], F32, name="psf")
                      for tch in range(2)}
                if USE_FP8_FFN:
                    for cp in range(NC // 2):
                        for tch in range(2):
                            nc.tensor.matmul(
                                pf[tch][:],
                                wfc_sb[:, 2 * cp:2 * cp + 2,
                                       fj * P:(fj + 1) * P],
                                xh2T[:, 2 * cp:2 * cp + 2,
                                     tch * 512:(tch + 1) * 512],
                                start=(cp == 0), stop=(cp == NC // 2 - 1),
                                perf_mode=DR)
                else:
                    for c in range(NC):
                        for tch in range(2):
                            nc.tensor.matmul(
                                pf[tch][:],
                                wfc_sb[:, c, fj * P:(fj + 1) * P],
                                xh2T[:, c, tch * 512:(tch + 1) * 512],
                                start=(c == 0), stop=(c == NC - 1))
                for tch in range(2):
                    nc.scalar.activation(
                        h2T[:, fj, tch * 512:(tch + 1) * 512], pf[tch][:],
                        AF.Gelu_apprx_tanh, bias=bfc_sb[:, fj:fj + 1],
                        scale=1.0 / WFC_SCALE if USE_FP8_FFN else 1.0)

        # ================= Phase G: FFN2 + residual + out =================
        with nc.named_scope("phG_ffn2"), ExitStack() as esG:
            psG = esG.enter_context(
                tc.tile_pool(name="psG", bufs=4, space="PSUM"))
            opool = esG.enter_context(tc.tile_pool(name="outp", bufs=3))
            for cc in range(2):
                if cc == 0:
                    wpj_t = wpj_t0
                else:
                    wpj_t = wpjp.tile([P, NF, 512], BF16, name="wpj_t")
                    nc.sync.dma_start(
                        wpj_t[:], wpj_r[:, :, cc * 512:(cc + 1) * 512])
                for qt in range(NTO):
                    pg = psG.tile([P, 512], F32, name="psg")
                    for f in range(NF):
                        nc.tensor.matmul(
                            pg[:], h2T[:, f, qt * P:(qt + 1) * P],
                            wpj_t[:, f, :],
                            start=(f == 0), stop=(f == NF - 1))
                    ot = opool.tile([P, 512], F32, name="ot")
                    nc.vector.tensor_tensor(
                        ot[:], pg[:],
                        x2[:, qt, cc * 512:(cc + 1) * 512], ALU.add)
                    nc.sync.dma_start(
                        out_d[qt * P:(qt + 1) * P, cc * 512:(cc + 1) * 512],
                        ot[:])

    nc.compile()
    return nc


def prepare_in_maps(x, ln1_g, ln1_b, w_qkv, b_qkv, w_attnproj, b_attnproj,
                    ln2_g, ln2_b, w_fc, b_fc, w_proj, b_proj):
    import ml_dtypes
    bf = ml_dtypes.bfloat16
    f8 = ml_dtypes.float8_e4m3

    x = np.asarray(x, np.float32)
    ln1_g = np.asarray(ln1_g, np.float32)
    ln1_b = np.asarray(ln1_b, np.float32)
    w_qkv = np.asarray(w_qkv, np.float32)
    b_qkv = np.asarray(b_qkv, np.float32)

    Wqkv = ln1_g[:, None] * w_qkv
    Bqkv = ln1_b @ w_qkv + b_qkv
    wq = np.ascontiguousarray(Wqkv[:, :C])
    wk = np.ascontiguousarray(Wqkv[:, C:2 * C])
    wv = np.ascontiguousarray(Wqkv[:, 2 * C:])
    bqk = np.concatenate([Bqkv[:C], Bqkv[C:2 * C]]).astype(np.float32)
    bv = Bqkv[2 * C:]
    assert np.all(bv == 0), "nonzero V bias not supported in this build"
    assert np.all(np.asarray(b_attnproj) == 0)
    assert np.all(np.asarray(b_proj) == 0)

    wfc = (np.asarray(ln2_g, np.float32)[:, None]
           * np.asarray(w_fc, np.float32))
    bfc = (np.asarray(ln2_b, np.float32) @ np.asarray(w_fc, np.float32)
           + np.asarray(b_fc, np.float32))
    wpj = np.asarray(w_proj, np.float32)

    if USE_FP8_FFN:
        wfc_c = (wfc * WFC_SCALE).astype(f8)
    else:
        wfc_c = wfc.astype(bf)
    wpj_c = wpj.astype(bf)

    shared = {
        "wq": wq.astype(bf), "wk": wk.astype(bf), "wv": wv.astype(bf),
        "bqk": bqk,
        "wap": np.asarray(w_attnproj, np.float32).astype(bf),
        "wfc": wfc_c,
        "bfc": bfc.astype(np.float32),
        "wpj": wpj_c,
    }
    in_maps = []
    for core in range(8):
        b, half = core // 2, core % 2
        xb = x[b]
        own = xb[half * TO:(half + 1) * TO]
        other = xb[(1 - half) * TO:(2 - half) * TO]
        m = dict(shared)
        m["x"] = np.ascontiguousarray(np.concatenate([own, other], 0))
        in_maps.append(m)
    return in_maps


def kernel(x, ln1_g, ln1_b, w_qkv, b_qkv, w_attnproj, b_attnproj,
           ln2_g, ln2_b, w_fc, b_fc, w_proj, b_proj):
    global LAST_RESULT
    in_maps = prepare_in_maps(
        x, ln1_g, ln1_b, w_qkv, b_qkv, w_attnproj, b_attnproj,
        ln2_g, ln2_b, w_fc, b_fc, w_proj, b_proj)

    if "nc" not in _CACHE:
        _CACHE["nc"] = _build()
    nc = _CACHE["nc"]

    LAST_RESULT = run_bass_kernel_spmd(nc, in_maps, core_ids=list(range(8)))

    out = np.empty((4, T, C), np.float32)
    for core in range(8):
        b, half = core // 2, core % 2
        out[b, half * TO:(half + 1) * TO] = LAST_RESULT.results[core]["out"]
    return out


# revision 11
# speedup vs baseline: 1.0461x; 1.0461x over previous
"""Trainium2 Bass kernel for a GPT-style transformer block.

B=4, T=2048, C=1024, H=16 heads (hd=64), D_FF=4096, fp32 I/O,
pre-LN, non-causal attention, tanh-approx GELU.

Sharding: 8 cores = 4 batch elements x 2 token-halves. Each core
computes attention K/V for its full batch element (dup of the K/V
projection for the other half -- avoids all collectives) and Q/MLP for
its own 1024 tokens. Host reorders tokens so each core's own tokens are
always rows 0..1023 -> identical NEFF on all 8 cores.

v2: bf16 activations/weights through attention (fast transposes, light
DMA), K/Q projection fused into the per-head-pair attention loop so exp
on the Activation engine overlaps PE work, stationary-reuse loop orders
(halved LDWEIGHTS), fp8e4m3 DoubleRow FFN (2 contraction rows per
partition), per-phase named scopes.
"""

import numpy as np
from contextlib import ExitStack

import concourse.bass as bass
import concourse.bacc as bacc
import concourse.mybir as mybir
from concourse import tile
from concourse.bass_utils import run_bass_kernel_spmd
from concourse.masks import make_identity

F32 = mybir.dt.float32
BF16 = mybir.dt.bfloat16
FP8 = mybir.dt.float8e4
AF = mybir.ActivationFunctionType
ALU = mybir.AluOpType
DR = mybir.MatmulPerfMode.DoubleRow

P = 128
T = 2048      # tokens per batch element (per core: kv tokens)
TO = 1024     # own tokens per core
C = 1024
H = 16
HD = 64
FF = 4096
NT = T // P   # 16 token tiles (kv)
NTO = TO // P  # 8 own token tiles
NC = C // P   # 8 channel tiles
NF = FF // P  # 32 ff tiles
EPS = 1e-5

USE_FP8_FFN = False
WFC_SCALE = 8.0
WPJ_SCALE = 16.0

_CACHE = {}
LAST_RESULT = None


def _ln_tile(nc, pools, src_ap, dstT, tslice, ident16, epsc):
    """LayerNorm one [128, C] token tile (gains folded into weights on
    host), cast to bf16, transpose into dstT[:, :, tslice] (dstT dtype)."""
    pool, spool, pps = pools
    st = spool.tile([P, 2, 6], F32, name="ln_st")
    for g in range(2):
        nc.vector.bn_stats(st[:, g], src_ap[:, g * 512:(g + 1) * 512])
    ag = spool.tile([P, 2], F32, name="ln_ag")
    nc.vector.bn_aggr(ag[:], st[:])
    std = spool.tile([P, 1], F32, name="ln_std")
    nc.scalar.activation(std[:], ag[:, 1:2], AF.Sqrt, bias=epsc)
    rinv = spool.tile([P, 1], F32, name="ln_rinv")
    nc.vector.reciprocal(rinv[:], std[:])
    xh = pool.tile([P, C], BF16, name="ln_xh")
    nc.vector.tensor_scalar(
        xh[:], src_ap, ag[:, 0:1], rinv[:], ALU.subtract, ALU.mult)
    for c in range(NC):
        tp = pps.tile([P, P], BF16, name="ln_tp")
        nc.tensor.transpose(tp[:], xh[:, c * P:(c + 1) * P], ident16)
        dst = dstT[:, c, tslice]
        if c % 2 == 0:
            nc.scalar.copy(dst, tp[:])
        else:
            nc.vector.tensor_copy(dst, tp[:])


def _build():
    nc = bacc.Bacc(None, target_bir_lowering=False)
    wdt = FP8 if USE_FP8_FFN else BF16

    # ---- DRAM I/O ----
    x_d = nc.dram_tensor("x", (T, C), F32, kind="ExternalInput")
    wq_d = nc.dram_tensor("wq", (C, C), BF16, kind="ExternalInput")
    wk_d = nc.dram_tensor("wk", (C, C), BF16, kind="ExternalInput")
    wv_d = nc.dram_tensor("wv", (C, C), BF16, kind="ExternalInput")
    bqk_d = nc.dram_tensor("bqk", (2 * C,), F32, kind="ExternalInput")
    wap_d = nc.dram_tensor("wap", (C, C), BF16, kind="ExternalInput")
    wfc_d = nc.dram_tensor("wfc", (C, FF), BF16, kind="ExternalInput")
    bfc_d = nc.dram_tensor("bfc", (FF,), F32, kind="ExternalInput")
    wpj_d = nc.dram_tensor("wpj", (FF, C), BF16, kind="ExternalInput")
    out_d = nc.dram_tensor("out", (TO, C), F32, kind="ExternalOutput")

    with tile.TileContext(nc) as tc, ExitStack() as top:
        cpool = top.enter_context(tc.tile_pool(name="const", bufs=1))
        epsc = cpool.tile([P, 1], F32, name="epsc")
        nc.vector.memset(epsc[:], EPS)
        ident16 = cpool.tile([P, P], BF16, name="ident16")
        make_identity(nc, ident16)
        ones_t = cpool.tile([P, HD], BF16, name="ones_t")
        nc.vector.memset(ones_t[:], 1.0)
        bqk_sb = cpool.tile([P, 2 * NC], F32, name="bqk_sb")
        nc.sync.dma_start(
            bqk_sb[:], bqk_d[:].rearrange("(j p) -> p j", p=P))
        bfc_sb = cpool.tile([P, NF], F32, name="bfc_sb")
        nc.sync.dma_start(
            bfc_sb[:], bfc_d[:].rearrange("(j p) -> p j", p=P))

        esA = top.enter_context(ExitStack())   # xhT (left)
        esBC = top.enter_context(ExitStack())  # vsb (right)
        esYW = top.enter_context(ExitStack())  # yT + wap: freed after D
        # yT lives from attention through phase D; allocate its pool below
        # const (before bigA) so left-side pool pops stay LIFO.
        yTp = esYW.enter_context(tc.tile_pool(name="yTp", bufs=1))

        # ============ Phase A: LN1 + transpose + V projection ============
        bigA = esA.enter_context(tc.tile_pool(name="bigA", bufs=1))
        xhT = bigA.tile([P, NC, T], BF16, name="xhT")  # 4 MB
        vsb = esBC.enter_context(
            tc.tile_pool(name="vsbp", bufs=1, side="right")).tile(
            [P, NT, H * (HD + 1)], BF16, name="vsb")
        with nc.named_scope("phA_ln_v"), ExitStack() as esAV:
            lnp = (esAV.enter_context(tc.tile_pool(name="ln_work", bufs=3)),
                   esAV.enter_context(tc.tile_pool(name="ln_stat", bufs=6)),
                   esAV.enter_context(
                       tc.tile_pool(name="ln_ps", bufs=2, space="PSUM")))
            xpool = esAV.enter_context(tc.tile_pool(name="xinp", bufs=3))
            wvp = esAV.enter_context(tc.tile_pool(name="wvp", bufs=1))
            psV = esAV.enter_context(
                tc.tile_pool(name="psV", bufs=4, space="PSUM"))
            wv_sb = wvp.tile([P, NC, C], BF16, name="wv_sb")
            nc.sync.dma_start(
                wv_sb[:], wv_d[:].rearrange("(c p) o -> p c o", p=P))
            for i in range(NT):
                xt = xpool.tile([P, C], F32, name="ln_x")
                nc.sync.dma_start(xt[:], x_d[i * P:(i + 1) * P, :])
                _ln_tile(nc, lnp, xt[:], xhT,
                         slice(i * P, (i + 1) * P), ident16, epsc[:])
                # V proj: stationary = token tile of xhT, reused across vc
                psv = {vc: psV.tile([P, 512], F32, name="psv")
                       for vc in range(2)}
                for c in range(NC):
                    for vc in range(2):
                        nc.tensor.matmul(
                            psv[vc][:], xhT[:, c, i * P:(i + 1) * P],
                            wv_sb[:, c, vc * 512:(vc + 1) * 512],
                            start=(c == 0), stop=(c == NC - 1))
                dstv = vsb[:, i].rearrange("p (h e) -> p h e", e=HD + 1)
                for vc in range(2):
                    nc.vector.tensor_copy(
                        dstv[:, vc * 8:(vc + 1) * 8, :HD],
                        psv[vc][:].rearrange("p (h d) -> p h d", d=HD))
                nc.gpsimd.memset(dstv[:, :, HD:], 1.0)

        # ========== Phase B+C fused: K/Q proj + attention per j ==========
        yT = yTp.tile([P, NC, TO], BF16, name="yT")
        with nc.named_scope("phBC_attn"), ExitStack() as esC:
            wkp = esC.enter_context(tc.tile_pool(name="wkp", bufs=2))
            wqp = esC.enter_context(tc.tile_pool(name="wqp", bufs=2))
            kTp = esC.enter_context(tc.tile_pool(name="kTp", bufs=2,
                                                 side="right"))
            qTp = esC.enter_context(tc.tile_pool(name="qTp", bufs=2,
                                                 side="right"))
            psKQ = esC.enter_context(
                tc.tile_pool(name="psKQ", bufs=2, space="PSUM"))
            psS = esC.enter_context(
                tc.tile_pool(name="psS", bufs=2, space="PSUM"))
            psO = esC.enter_context(
                tc.tile_pool(name="psO", bufs=2, space="PSUM"))
            ppool = esC.enter_context(tc.tile_pool(name="pT", bufs=33))
            scrp = esC.enter_context(tc.tile_pool(name="scrp", bufs=2))
            dflp = esC.enter_context(tc.tile_pool(name="dflp", bufs=2))
            dinp = esC.enter_context(tc.tile_pool(name="dinp", bufs=2))
            wk_r = wk_d[:].rearrange("(c p) o -> p c o", p=P)
            wq_r = wq_d[:].rearrange("(c p) o -> p c o", p=P)
            for j in range(H // 2):
                wk_t = wkp.tile([P, NC, P], BF16, name="wk_t")
                nc.sync.dma_start(wk_t[:], wk_r[:, :, j * P:(j + 1) * P])
                wq_t = wqp.tile([P, NC, P], BF16, name="wq_t")
                nc.sync.dma_start(wq_t[:], wq_r[:, :, j * P:(j + 1) * P])
                kTj = kTp.tile([P, T], BF16, name="kTj")
                qTj = qTp.tile([P, TO], BF16, name="qTj")
                # K projection: stationary reused across 2 token chunks
                for tg in range(2):
                    pk = {t2: psKQ.tile([P, 512], F32, name="pskq")
                          for t2 in range(2)}
                    for c in range(NC):
                        for t2 in range(2):
                            toff = tg * 1024 + t2 * 512
                            nc.tensor.matmul(
                                pk[t2][:], wk_t[:, c],
                                xhT[:, c, toff:toff + 512],
                                start=(c == 0), stop=(c == NC - 1))
                    for t2 in range(2):
                        toff = tg * 1024 + t2 * 512
                        nc.vector.tensor_scalar_add(
                            kTj[:, toff:toff + 512], pk[t2][:],
                            bqk_sb[:, NC + j:NC + j + 1])
                # Q projection (own tokens only)
                pq = {t2: psKQ.tile([P, 512], F32, name="pskq")
                      for t2 in range(2)}
                for c in range(NC):
                    for t2 in range(2):
                        nc.tensor.matmul(
                            pq[t2][:], wq_t[:, c],
                            xhT[:, c, t2 * 512:(t2 + 1) * 512],
                            start=(c == 0), stop=(c == NC - 1))
                for t2 in range(2):
                    nc.vector.tensor_scalar_add(
                        qTj[:, t2 * 512:(t2 + 1) * 512], pq[t2][:],
                        bqk_sb[:, j:j + 1])
                # scores + exp; even/odd head pair on PE quadrants 0/64
                pT = {0: [None] * NT, 64: [None] * NT}
                for k in range(NT):
                    sps = {po: psS.tile([P, TO], F32, name="sps", tag="sps")
                           for po in (0, 64)}
                    for qc in range(2):
                        for po in (0, 64):
                            nc.tensor.matmul(
                                sps[po][:, qc * 512:(qc + 1) * 512],
                                kTj[po:po + HD, k * P:(k + 1) * P],
                                qTj[po:po + HD, qc * 512:(qc + 1) * 512],
                                start=True, stop=True)
                    for po in (0, 64):
                        pT[po][k] = ppool.tile([P, TO], BF16, name="pT_t")
                        nc.scalar.activation(
                            pT[po][k][:], sps[po][:], AF.Exp, scale=0.125)
                # wide PV: out = [V | 1]^T @ P -> [65, 512] = y^T rows +
                # denominator row; then divide via PE ones-broadcast of 1/d
                for po in (0, 64):
                    h = 2 * j + (po // HD)
                    opses = []
                    for qg in range(2):
                        ops = psO.tile([P, 512], F32, name="ops")
                        for k in range(NT):
                            nc.tensor.matmul(
                                ops[:HD + 1, :],
                                vsb[:, k, h * (HD + 1):(h + 1) * (HD + 1)],
                                pT[po][k][:, qg * 512:(qg + 1) * 512],
                                start=(k == 0), stop=(k == NT - 1))
                        scr = scrp.tile([P, 512], BF16, name="scr")
                        nc.vector.tensor_copy(
                            scr[HD:HD + 1, :], ops[HD:HD + 1, :])
                        opses.append((ops, scr))
                    # denom row (partition 64) -> broadcast to 64 partitions
                    # via a 1-row matmul on PE row-tile 64; then 1/d and scale
                    for qg, (ops, scr) in enumerate(opses):
                        dps = psKQ.tile([P, 512], F32, name="pskq")
                        nc.tensor.matmul(
                            dps[:HD, :], ones_t[HD:HD + 1, :],
                            scr[HD:HD + 1, :], start=True, stop=True,
                            tile_position=(HD, 0))
                        dinvT = dinp.tile([HD, 512], BF16, name="dinvT")
                        with nc.allow_low_precision(
                                reason="bf16 1/denom is plenty for softmax"):
                            nc.vector.reciprocal(dinvT[:], dps[:HD, :])
                        nc.vector.tensor_tensor(
                            yT[po:po + HD, j, qg * 512:(qg + 1) * 512],
                            ops[:HD, :], dinvT[:], ALU.mult)
        esA.close()   # free xhT
        esBC.close()  # free vsb

        # ========== Phase D: attn proj + residual + LN2 fused ==========
        x2 = top.enter_context(
            tc.tile_pool(name="x2p", bufs=1, side="right")).tile(
            [P, NTO, C], F32, name="x2")
        xh2T = top.enter_context(
            tc.tile_pool(name="bigE", bufs=1, side="right")).tile(
            [P, NC, TO], wdt, name="xh2T")
        wap_sb = esYW.enter_context(
            tc.tile_pool(name="wapp", bufs=1)).tile(
            [P, NC, C], BF16, name="wap_sb")
        nc.sync.dma_start(
            wap_sb[:], wap_d[:].rearrange("(c p) o -> p c o", p=P))
        # h2T (bf16, FFN1 out / FFN2 stationary); wfc streamed in halves
        h2T = top.enter_context(
            tc.tile_pool(name="h2Tp", bufs=1, side="right")).tile(
            [P, NF, TO], BF16, name="h2T")
        wfcp = top.enter_context(
            tc.tile_pool(name="wfcp", bufs=1, side="right"))
        wfc_r = wfc_d[:].rearrange("(c p) f -> p c f", p=P)
        wfc_t0 = wfcp.tile([P, NC, FF // 2], BF16, name="wfc_t")
        nc.sync.dma_start(wfc_t0[:], wfc_r[:, :, 0:FF // 2])
        with nc.named_scope("phD_proj_ln2"), ExitStack() as esD:
            xrp = esD.enter_context(tc.tile_pool(name="xrp", bufs=3))
            psD = esD.enter_context(
                tc.tile_pool(name="psD", bufs=4, space="PSUM"))
            ln2p = (esD.enter_context(tc.tile_pool(name="ln2_work", bufs=3)),
                    esD.enter_context(tc.tile_pool(name="ln2_stat", bufs=6)),
                    esD.enter_context(
                        tc.tile_pool(name="ln2_ps", bufs=2, space="PSUM")))
            for qt in range(NTO):
                xr = xrp.tile([P, C], F32, name="xr")
                nc.sync.dma_start(xr[:], x_d[qt * P:(qt + 1) * P, :])
                pd = {cc: psD.tile([P, 512], F32, name="psd")
                      for cc in range(2)}
                for c in range(NC):
                    for cc in range(2):
                        nc.tensor.matmul(
                            pd[cc][:], yT[:, c, qt * P:(qt + 1) * P],
                            wap_sb[:, c, cc * 512:(cc + 1) * 512],
                            start=(c == 0), stop=(c == NC - 1))
                for cc in range(2):
                    nc.vector.tensor_tensor(
                        x2[:, qt, cc * 512:(cc + 1) * 512], pd[cc][:],
                        xr[:, cc * 512:(cc + 1) * 512], ALU.add)
                _ln_tile(nc, ln2p, x2[:, qt], xh2T,
                         slice(qt * P, (qt + 1) * P), ident16, epsc[:])

        # ================= Phase F: FFN1 + gelu =================
        esYW.close()  # free yT, wap
        wpjp = top.enter_context(
            tc.tile_pool(name="wpjp", bufs=1, side="right"))
        wpj_r = wpj_d[:].rearrange("(f p) o -> p f o", p=P)
        wpj_t0 = wpjp.tile([P, NF, 512], BF16, name="wpj_t")
        nc.sync.dma_start(wpj_t0[:], wpj_r[:, :, 0:512])
        with nc.named_scope("phF_ffn1"), ExitStack() as esF:
            psF = esF.enter_context(
                tc.tile_pool(name="psF", bufs=4, space="PSUM"))
            for fh in range(2):
                if fh == 0:
                    wfc_t = wfc_t0
                else:
                    wfc_t = wfcp.tile([P, NC, FF // 2], BF16, name="wfc_t")
                    nc.sync.dma_start(
                        wfc_t[:], wfc_r[:, :, FF // 2:FF])
                for fj in range(NF // 2):
                    fjg = fh * (NF // 2) + fj
                    pf = {tch: psF.tile([P, 512], F32, name="psf")
                          for tch in range(2)}
                    for c in range(NC):
                        for tch in range(2):
                            nc.tensor.matmul(
                                pf[tch][:],
                                wfc_t[:, c, fj * P:(fj + 1) * P],
                                xh2T[:, c, tch * 512:(tch + 1) * 512],
                                start=(c == 0), stop=(c == NC - 1))
                    for tch in range(2):
                        nc.scalar.activation(
                            h2T[:, fjg, tch * 512:(tch + 1) * 512],
                            pf[tch][:], AF.Gelu_apprx_tanh,
                            bias=bfc_sb[:, fjg:fjg + 1])

        # ================= Phase G: FFN2 + residual + out =================
        with nc.named_scope("phG_ffn2"), ExitStack() as esG:
            psG = esG.enter_context(
                tc.tile_pool(name="psG", bufs=4, space="PSUM"))
            opool = esG.enter_context(tc.tile_pool(name="outp", bufs=3))
            for cc in range(2):
                if cc == 0:
                    wpj_t = wpj_t0
                else:
                    wpj_t = wpjp.tile([P, NF, 512], BF16, name="wpj_t")
                    nc.sync.dma_start(
                        wpj_t[:], wpj_r[:, :, cc * 512:(cc + 1) * 512])
                for qt in range(NTO):
                    pg = psG.tile([P, 512], F32, name="psg")
                    for f in range(NF):
                        nc.tensor.matmul(
                            pg[:], h2T[:, f, qt * P:(qt + 1) * P],
                            wpj_t[:, f, :],
                            start=(f == 0), stop=(f == NF - 1))
                    ot = opool.tile([P, 512], F32, name="ot")
                    nc.vector.tensor_tensor(
                        ot[:], pg[:],
                        x2[:, qt, cc * 512:(cc + 1) * 512], ALU.add)
                    nc.sync.dma_start(
                        out_d[qt * P:(qt + 1) * P, cc * 512:(cc + 1) * 512],
                        ot[:])

    nc.compile()
    return nc


def prepare_in_maps(x, ln1_g, ln1_b, w_qkv, b_qkv, w_attnproj, b_attnproj,
                    ln2_g, ln2_b, w_fc, b_fc, w_proj, b_proj):
    import ml_dtypes
    bf = ml_dtypes.bfloat16
    f8 = ml_dtypes.float8_e4m3

    x = np.asarray(x, np.float32)
    ln1_g = np.asarray(ln1_g, np.float32)
    ln1_b = np.asarray(ln1_b, np.float32)
    w_qkv = np.asarray(w_qkv, np.float32)
    b_qkv = np.asarray(b_qkv, np.float32)

    Wqkv = ln1_g[:, None] * w_qkv
    Bqkv = ln1_b @ w_qkv + b_qkv
    wq = np.ascontiguousarray(Wqkv[:, :C])
    wk = np.ascontiguousarray(Wqkv[:, C:2 * C])
    wv = np.ascontiguousarray(Wqkv[:, 2 * C:])
    bqk = np.concatenate([Bqkv[:C], Bqkv[C:2 * C]]).astype(np.float32)
    bv = Bqkv[2 * C:]
    assert np.all(bv == 0), "nonzero V bias not supported in this build"
    assert np.all(np.asarray(b_attnproj) == 0)
    assert np.all(np.asarray(b_proj) == 0)

    wfc = (np.asarray(ln2_g, np.float32)[:, None]
           * np.asarray(w_fc, np.float32))
    bfc = (np.asarray(ln2_b, np.float32) @ np.asarray(w_fc, np.float32)
           + np.asarray(b_fc, np.float32))
    wpj = np.asarray(w_proj, np.float32)

    wfc_c = wfc.astype(bf)
    wpj_c = wpj.astype(bf)

    shared = {
        "wq": wq.astype(bf), "wk": wk.astype(bf), "wv": wv.astype(bf),
        "bqk": bqk,
        "wap": np.asarray(w_attnproj, np.float32).astype(bf),
        "wfc": wfc_c,
        "bfc": bfc.astype(np.float32),
        "wpj": wpj_c,
    }
    in_maps = []
    for core in range(8):
        b, half = core // 2, core % 2
        xb = x[b]
        own = xb[half * TO:(half + 1) * TO]
        other = xb[(1 - half) * TO:(2 - half) * TO]
        m = dict(shared)
        m["x"] = np.ascontiguousarray(np.concatenate([own, other], 0))
        in_maps.append(m)
    return in_maps


def kernel(x, ln1_g, ln1_b, w_qkv, b_qkv, w_attnproj, b_attnproj,
           ln2_g, ln2_b, w_fc, b_fc, w_proj, b_proj):
    global LAST_RESULT
    in_maps = prepare_in_maps(
        x, ln1_g, ln1_b, w_qkv, b_qkv, w_attnproj, b_attnproj,
        ln2_g, ln2_b, w_fc, b_fc, w_proj, b_proj)

    if "nc" not in _CACHE:
        _CACHE["nc"] = _build()
    nc = _CACHE["nc"]

    LAST_RESULT = run_bass_kernel_spmd(nc, in_maps, core_ids=list(range(8)))

    out = np.empty((4, T, C), np.float32)
    for core in range(8):
        b, half = core // 2, core % 2
        out[b, half * TO:(half + 1) * TO] = LAST_RESULT.results[core]["out"]
    return out


# revision 12
# speedup vs baseline: 1.0483x; 1.0021x over previous
"""Trainium2 Bass kernel for a GPT-style transformer block.

B=4, T=2048, C=1024, H=16 heads (hd=64), D_FF=4096, fp32 I/O,
pre-LN, non-causal attention, tanh-approx GELU.

Sharding: 8 cores = 4 batch elements x 2 token-halves. Each core
computes attention K/V for its full batch element (dup of the K/V
projection for the other half -- avoids all collectives) and Q/MLP for
its own 1024 tokens. Host reorders tokens so each core's own tokens are
always rows 0..1023 -> identical NEFF on all 8 cores.

v2: bf16 activations/weights through attention (fast transposes, light
DMA), K/Q projection fused into the per-head-pair attention loop so exp
on the Activation engine overlaps PE work, stationary-reuse loop orders
(halved LDWEIGHTS), fp8e4m3 DoubleRow FFN (2 contraction rows per
partition), per-phase named scopes.
"""

import numpy as np
from contextlib import ExitStack

import concourse.bass as bass
import concourse.bacc as bacc
import concourse.mybir as mybir
from concourse import tile
from concourse.bass_utils import run_bass_kernel_spmd
from concourse.masks import make_identity

F32 = mybir.dt.float32
BF16 = mybir.dt.bfloat16
FP8 = mybir.dt.float8e4
AF = mybir.ActivationFunctionType
ALU = mybir.AluOpType
DR = mybir.MatmulPerfMode.DoubleRow

P = 128
T = 2048      # tokens per batch element (per core: kv tokens)
TO = 1024     # own tokens per core
C = 1024
H = 16
HD = 64
FF = 4096
NT = T // P   # 16 token tiles (kv)
NTO = TO // P  # 8 own token tiles
NC = C // P   # 8 channel tiles
NF = FF // P  # 32 ff tiles
EPS = 1e-5

USE_FP8_FFN = False
WFC_SCALE = 8.0
WPJ_SCALE = 16.0

_CACHE = {}
LAST_RESULT = None


def _ln_tile(nc, pools, src_ap, dstT, tslice, ident16, epsc):
    """LayerNorm one [128, C] token tile (gains folded into weights on
    host), cast to bf16, transpose into dstT[:, :, tslice] (dstT dtype)."""
    pool, spool, pps = pools
    st = spool.tile([P, 2, 6], F32, name="ln_st")
    for g in range(2):
        nc.vector.bn_stats(st[:, g], src_ap[:, g * 512:(g + 1) * 512])
    ag = spool.tile([P, 2], F32, name="ln_ag")
    nc.vector.bn_aggr(ag[:], st[:])
    std = spool.tile([P, 1], F32, name="ln_std")
    nc.scalar.activation(std[:], ag[:, 1:2], AF.Sqrt, bias=epsc)
    rinv = spool.tile([P, 1], F32, name="ln_rinv")
    nc.vector.reciprocal(rinv[:], std[:])
    xh = pool.tile([P, C], BF16, name="ln_xh")
    nc.vector.tensor_scalar(
        xh[:], src_ap, ag[:, 0:1], rinv[:], ALU.subtract, ALU.mult)
    for c in range(NC):
        tp = pps.tile([P, P], BF16, name="ln_tp")
        nc.tensor.transpose(tp[:], xh[:, c * P:(c + 1) * P], ident16)
        dst = dstT[:, c, tslice]
        if c % 2 == 0:
            nc.scalar.copy(dst, tp[:])
        else:
            nc.vector.tensor_copy(dst, tp[:])


def _build():
    nc = bacc.Bacc(None, target_bir_lowering=False)
    wdt = FP8 if USE_FP8_FFN else BF16

    # ---- DRAM I/O ----
    x_d = nc.dram_tensor("x", (T, C), F32, kind="ExternalInput")
    wq_d = nc.dram_tensor("wq", (C, C), BF16, kind="ExternalInput")
    wk_d = nc.dram_tensor("wk", (C, C), BF16, kind="ExternalInput")
    wv_d = nc.dram_tensor("wv", (C, C), BF16, kind="ExternalInput")
    bqk_d = nc.dram_tensor("bqk", (2 * C,), F32, kind="ExternalInput")
    wap_d = nc.dram_tensor("wap", (C, C), BF16, kind="ExternalInput")
    wfc_d = nc.dram_tensor("wfc", (C, FF), BF16, kind="ExternalInput")
    bfc_d = nc.dram_tensor("bfc", (FF,), F32, kind="ExternalInput")
    wpj_d = nc.dram_tensor("wpj", (FF, C), BF16, kind="ExternalInput")
    out_d = nc.dram_tensor("out", (TO, C), F32, kind="ExternalOutput")

    with tile.TileContext(nc) as tc, ExitStack() as top:
        cpool = top.enter_context(tc.tile_pool(name="const", bufs=1))
        epsc = cpool.tile([P, 1], F32, name="epsc")
        nc.vector.memset(epsc[:], EPS)
        ident16 = cpool.tile([P, P], BF16, name="ident16")
        make_identity(nc, ident16)
        ones_t = cpool.tile([P, HD], BF16, name="ones_t")
        nc.vector.memset(ones_t[:], 1.0)
        bqk_sb = cpool.tile([P, 2 * NC], F32, name="bqk_sb")
        nc.sync.dma_start(
            bqk_sb[:], bqk_d[:].rearrange("(j p) -> p j", p=P))
        bfc_sb = cpool.tile([P, NF], F32, name="bfc_sb")
        nc.sync.dma_start(
            bfc_sb[:], bfc_d[:].rearrange("(j p) -> p j", p=P))

        esA = top.enter_context(ExitStack())   # xhT (left)
        esBC = top.enter_context(ExitStack())  # vsb (right)
        esYW = top.enter_context(ExitStack())  # yT + wap: freed after D
        # yT lives from attention through phase D; allocate its pool below
        # const (before bigA) so left-side pool pops stay LIFO.
        yTp = esYW.enter_context(tc.tile_pool(name="yTp", bufs=1))

        # ============ Phase A: LN1 + transpose + V projection ============
        bigA = esA.enter_context(tc.tile_pool(name="bigA", bufs=1))
        xhT = bigA.tile([P, NC, T], BF16, name="xhT")  # 4 MB
        vsb = esBC.enter_context(
            tc.tile_pool(name="vsbp", bufs=1, side="right")).tile(
            [P, NT, H * (HD + 1)], BF16, name="vsb")
        with nc.named_scope("phA_ln_v"), ExitStack() as esAV:
            lnp = (esAV.enter_context(tc.tile_pool(name="ln_work", bufs=3)),
                   esAV.enter_context(tc.tile_pool(name="ln_stat", bufs=6)),
                   esAV.enter_context(
                       tc.tile_pool(name="ln_ps", bufs=2, space="PSUM")))
            xpool = esAV.enter_context(tc.tile_pool(name="xinp", bufs=3))
            wvp = esAV.enter_context(tc.tile_pool(name="wvp", bufs=1))
            psV = esAV.enter_context(
                tc.tile_pool(name="psV", bufs=4, space="PSUM"))
            wv_sb = wvp.tile([P, NC, C], BF16, name="wv_sb")
            nc.sync.dma_start(
                wv_sb[:], wv_d[:].rearrange("(c p) o -> p c o", p=P))
            for i in range(NT):
                xt = xpool.tile([P, C], F32, name="ln_x")
                nc.sync.dma_start(xt[:], x_d[i * P:(i + 1) * P, :])
                _ln_tile(nc, lnp, xt[:], xhT,
                         slice(i * P, (i + 1) * P), ident16, epsc[:])
                # V proj: stationary = token tile of xhT, reused across vc
                psv = {vc: psV.tile([P, 512], F32, name="psv")
                       for vc in range(2)}
                for c in range(NC):
                    for vc in range(2):
                        nc.tensor.matmul(
                            psv[vc][:], xhT[:, c, i * P:(i + 1) * P],
                            wv_sb[:, c, vc * 512:(vc + 1) * 512],
                            start=(c == 0), stop=(c == NC - 1))
                dstv = vsb[:, i].rearrange("p (h e) -> p h e", e=HD + 1)
                for vc in range(2):
                    nc.vector.tensor_copy(
                        dstv[:, vc * 8:(vc + 1) * 8, :HD],
                        psv[vc][:].rearrange("p (h d) -> p h d", d=HD))
                nc.gpsimd.memset(dstv[:, :, HD:], 1.0)

        # ========== Phase B+C fused: K/Q proj + attention per j ==========
        yT = yTp.tile([P, NC, TO], BF16, name="yT")
        with nc.named_scope("phBC_attn"), ExitStack() as esC:
            wkp = esC.enter_context(tc.tile_pool(name="wkp", bufs=2))
            wqp = esC.enter_context(tc.tile_pool(name="wqp", bufs=2))
            kTp = esC.enter_context(tc.tile_pool(name="kTp", bufs=2,
                                                 side="right"))
            qTp = esC.enter_context(tc.tile_pool(name="qTp", bufs=2,
                                                 side="right"))
            psKQ = esC.enter_context(
                tc.tile_pool(name="psKQ", bufs=2, space="PSUM"))
            psS = esC.enter_context(
                tc.tile_pool(name="psS", bufs=2, space="PSUM"))
            psO = esC.enter_context(
                tc.tile_pool(name="psO", bufs=2, space="PSUM"))
            ppool = esC.enter_context(tc.tile_pool(name="pT", bufs=33))
            scrp = esC.enter_context(tc.tile_pool(name="scrp", bufs=2))
            dflp = esC.enter_context(tc.tile_pool(name="dflp", bufs=2))
            dinp = esC.enter_context(tc.tile_pool(name="dinp", bufs=2))
            wk_r = wk_d[:].rearrange("(c p) o -> p c o", p=P)
            wq_r = wq_d[:].rearrange("(c p) o -> p c o", p=P)
            for j in range(H // 2):
                wk_t = wkp.tile([P, NC, P], BF16, name="wk_t")
                nc.sync.dma_start(wk_t[:], wk_r[:, :, j * P:(j + 1) * P])
                wq_t = wqp.tile([P, NC, P], BF16, name="wq_t")
                nc.sync.dma_start(wq_t[:], wq_r[:, :, j * P:(j + 1) * P])
                kTj = kTp.tile([P, T], BF16, name="kTj")
                qTj = qTp.tile([P, TO], BF16, name="qTj")
                # K projection: stationary reused across 2 token chunks
                for tg in range(2):
                    pk = {t2: psKQ.tile([P, 512], F32, name="pskq")
                          for t2 in range(2)}
                    for c in range(NC):
                        for t2 in range(2):
                            toff = tg * 1024 + t2 * 512
                            nc.tensor.matmul(
                                pk[t2][:], wk_t[:, c],
                                xhT[:, c, toff:toff + 512],
                                start=(c == 0), stop=(c == NC - 1))
                    for t2 in range(2):
                        toff = tg * 1024 + t2 * 512
                        nc.vector.tensor_scalar_add(
                            kTj[:, toff:toff + 512], pk[t2][:],
                            bqk_sb[:, NC + j:NC + j + 1])
                # Q projection (own tokens only)
                pq = {t2: psKQ.tile([P, 512], F32, name="pskq")
                      for t2 in range(2)}
                for c in range(NC):
                    for t2 in range(2):
                        nc.tensor.matmul(
                            pq[t2][:], wq_t[:, c],
                            xhT[:, c, t2 * 512:(t2 + 1) * 512],
                            start=(c == 0), stop=(c == NC - 1))
                for t2 in range(2):
                    nc.vector.tensor_scalar_add(
                        qTj[:, t2 * 512:(t2 + 1) * 512], pq[t2][:],
                        bqk_sb[:, j:j + 1])
                # scores + exp; even/odd head pair on PE quadrants 0/64
                pT = {0: [None] * NT, 64: [None] * NT}
                for k in range(NT):
                    sps = {po: psS.tile([P, TO], F32, name="sps", tag="sps")
                           for po in (0, 64)}
                    for qc in range(2):
                        for po in (0, 64):
                            nc.tensor.matmul(
                                sps[po][:, qc * 512:(qc + 1) * 512],
                                kTj[po:po + HD, k * P:(k + 1) * P],
                                qTj[po:po + HD, qc * 512:(qc + 1) * 512],
                                start=True, stop=True)
                    for po in (0, 64):
                        pT[po][k] = ppool.tile([P, TO], BF16, name="pT_t")
                        nc.scalar.activation(
                            pT[po][k][:], sps[po][:], AF.Exp, scale=0.125)
                # wide PV: out = [V | 1]^T @ P -> [65, 512] = y^T rows +
                # denominator row; then divide via PE ones-broadcast of 1/d
                for po in (0, 64):
                    h = 2 * j + (po // HD)
                    # two interleaved chains (alternating PSUM banks hides
                    # the accumulation turnaround; vsb stationary reused 2x)
                    opses = []
                    ops2 = {qg: psO.tile([P, 512], F32, name="ops")
                            for qg in range(2)}
                    for k in range(NT):
                        for qg in range(2):
                            nc.tensor.matmul(
                                ops2[qg][:HD + 1, :],
                                vsb[:, k, h * (HD + 1):(h + 1) * (HD + 1)],
                                pT[po][k][:, qg * 512:(qg + 1) * 512],
                                start=(k == 0), stop=(k == NT - 1))
                    for qg in range(2):
                        scr = scrp.tile([P, 512], BF16, name="scr")
                        nc.vector.tensor_copy(
                            scr[HD:HD + 1, :], ops2[qg][HD:HD + 1, :])
                        opses.append((ops2[qg], scr))
                    # denom row (partition 64) -> broadcast to 64 partitions
                    # via a 1-row matmul on PE row-tile 64; then 1/d and scale
                    for qg, (ops, scr) in enumerate(opses):
                        dps = psKQ.tile([P, 512], F32, name="pskq")
                        nc.tensor.matmul(
                            dps[:HD, :], ones_t[HD:HD + 1, :],
                            scr[HD:HD + 1, :], start=True, stop=True,
                            tile_position=(HD, 0))
                        dinvT = dinp.tile([HD, 512], BF16, name="dinvT")
                        with nc.allow_low_precision(
                                reason="bf16 1/denom is plenty for softmax"):
                            nc.vector.reciprocal(dinvT[:], dps[:HD, :])
                        nc.vector.tensor_tensor(
                            yT[po:po + HD, j, qg * 512:(qg + 1) * 512],
                            ops[:HD, :], dinvT[:], ALU.mult)
        esA.close()   # free xhT
        esBC.close()  # free vsb

        # ========== Phase D: attn proj + residual + LN2 fused ==========
        x2 = top.enter_context(
            tc.tile_pool(name="x2p", bufs=1, side="right")).tile(
            [P, NTO, C], F32, name="x2")
        xh2T = top.enter_context(
            tc.tile_pool(name="bigE", bufs=1, side="right")).tile(
            [P, NC, TO], wdt, name="xh2T")
        wap_sb = esYW.enter_context(
            tc.tile_pool(name="wapp", bufs=1)).tile(
            [P, NC, C], BF16, name="wap_sb")
        nc.sync.dma_start(
            wap_sb[:], wap_d[:].rearrange("(c p) o -> p c o", p=P))
        # h2T (bf16, FFN1 out / FFN2 stationary); wfc streamed in halves
        h2T = top.enter_context(
            tc.tile_pool(name="h2Tp", bufs=1, side="right")).tile(
            [P, NF, TO], BF16, name="h2T")
        wfcp = top.enter_context(
            tc.tile_pool(name="wfcp", bufs=1, side="right"))
        wfc_r = wfc_d[:].rearrange("(c p) f -> p c f", p=P)
        wfc_t0 = wfcp.tile([P, NC, FF // 2], BF16, name="wfc_t")
        nc.sync.dma_start(wfc_t0[:], wfc_r[:, :, 0:FF // 2])
        with nc.named_scope("phD_proj_ln2"), ExitStack() as esD:
            xrp = esD.enter_context(tc.tile_pool(name="xrp", bufs=3))
            psD = esD.enter_context(
                tc.tile_pool(name="psD", bufs=4, space="PSUM"))
            ln2p = (esD.enter_context(tc.tile_pool(name="ln2_work", bufs=3)),
                    esD.enter_context(tc.tile_pool(name="ln2_stat", bufs=6)),
                    esD.enter_context(
                        tc.tile_pool(name="ln2_ps", bufs=2, space="PSUM")))
            for qt in range(NTO):
                xr = xrp.tile([P, C], F32, name="xr")
                nc.sync.dma_start(xr[:], x_d[qt * P:(qt + 1) * P, :])
                pd = {cc: psD.tile([P, 512], F32, name="psd")
                      for cc in range(2)}
                for c in range(NC):
                    for cc in range(2):
                        nc.tensor.matmul(
                            pd[cc][:], yT[:, c, qt * P:(qt + 1) * P],
                            wap_sb[:, c, cc * 512:(cc + 1) * 512],
                            start=(c == 0), stop=(c == NC - 1))
                for cc in range(2):
                    nc.vector.tensor_tensor(
                        x2[:, qt, cc * 512:(cc + 1) * 512], pd[cc][:],
                        xr[:, cc * 512:(cc + 1) * 512], ALU.add)
                _ln_tile(nc, ln2p, x2[:, qt], xh2T,
                         slice(qt * P, (qt + 1) * P), ident16, epsc[:])

        # ================= Phase F: FFN1 + gelu =================
        esYW.close()  # free yT, wap
        wpjp = top.enter_context(
            tc.tile_pool(name="wpjp", bufs=1, side="right"))
        wpj_r = wpj_d[:].rearrange("(f p) o -> p f o", p=P)
        wpj_t0 = wpjp.tile([P, NF, 512], BF16, name="wpj_t")
        nc.sync.dma_start(wpj_t0[:], wpj_r[:, :, 0:512])
        with nc.named_scope("phF_ffn1"), ExitStack() as esF:
            psF = esF.enter_context(
                tc.tile_pool(name="psF", bufs=4, space="PSUM"))
            for fh in range(2):
                if fh == 0:
                    wfc_t = wfc_t0
                else:
                    wfc_t = wfcp.tile([P, NC, FF // 2], BF16, name="wfc_t")
                    nc.sync.dma_start(
                        wfc_t[:], wfc_r[:, :, FF // 2:FF])
                for fj in range(NF // 2):
                    fjg = fh * (NF // 2) + fj
                    pf = {tch: psF.tile([P, 512], F32, name="psf")
                          for tch in range(2)}
                    for c in range(NC):
                        for tch in range(2):
                            nc.tensor.matmul(
                                pf[tch][:],
                                wfc_t[:, c, fj * P:(fj + 1) * P],
                                xh2T[:, c, tch * 512:(tch + 1) * 512],
                                start=(c == 0), stop=(c == NC - 1))
                    for tch in range(2):
                        nc.scalar.activation(
                            h2T[:, fjg, tch * 512:(tch + 1) * 512],
                            pf[tch][:], AF.Gelu_apprx_tanh,
                            bias=bfc_sb[:, fjg:fjg + 1])

        # ================= Phase G: FFN2 + residual + out =================
        with nc.named_scope("phG_ffn2"), ExitStack() as esG:
            psG = esG.enter_context(
                tc.tile_pool(name="psG", bufs=4, space="PSUM"))
            opool = esG.enter_context(tc.tile_pool(name="outp", bufs=3))
            for cc in range(2):
                if cc == 0:
                    wpj_t = wpj_t0
                else:
                    wpj_t = wpjp.tile([P, NF, 512], BF16, name="wpj_t")
                    nc.sync.dma_start(
                        wpj_t[:], wpj_r[:, :, cc * 512:(cc + 1) * 512])
                for qt in range(NTO):
                    pg = psG.tile([P, 512], F32, name="psg")
                    for f in range(NF):
                        nc.tensor.matmul(
                            pg[:], h2T[:, f, qt * P:(qt + 1) * P],
                            wpj_t[:, f, :],
                            start=(f == 0), stop=(f == NF - 1))
                    ot = opool.tile([P, 512], F32, name="ot")
                    nc.vector.tensor_tensor(
                        ot[:], pg[:],
                        x2[:, qt, cc * 512:(cc + 1) * 512], ALU.add)
                    nc.sync.dma_start(
                        out_d[qt * P:(qt + 1) * P, cc * 512:(cc + 1) * 512],
                        ot[:])

    nc.compile()
    return nc


def prepare_in_maps(x, ln1_g, ln1_b, w_qkv, b_qkv, w_attnproj, b_attnproj,
                    ln2_g, ln2_b, w_fc, b_fc, w_proj, b_proj):
    import ml_dtypes
    bf = ml_dtypes.bfloat16
    f8 = ml_dtypes.float8_e4m3

    x = np.asarray(x, np.float32)
    ln1_g = np.asarray(ln1_g, np.float32)
    ln1_b = np.asarray(ln1_b, np.float32)
    w_qkv = np.asarray(w_qkv, np.float32)
    b_qkv = np.asarray(b_qkv, np.float32)

    Wqkv = ln1_g[:, None] * w_qkv
    Bqkv = ln1_b @ w_qkv + b_qkv
    wq = np.ascontiguousarray(Wqkv[:, :C])
    wk = np.ascontiguousarray(Wqkv[:, C:2 * C])
    wv = np.ascontiguousarray(Wqkv[:, 2 * C:])
    bqk = np.concatenate([Bqkv[:C], Bqkv[C:2 * C]]).astype(np.float32)
    bv = Bqkv[2 * C:]
    assert np.all(bv == 0), "nonzero V bias not supported in this build"
    assert np.all(np.asarray(b_attnproj) == 0)
    assert np.all(np.asarray(b_proj) == 0)

    wfc = (np.asarray(ln2_g, np.float32)[:, None]
           * np.asarray(w_fc, np.float32))
    bfc = (np.asarray(ln2_b, np.float32) @ np.asarray(w_fc, np.float32)
           + np.asarray(b_fc, np.float32))
    wpj = np.asarray(w_proj, np.float32)

    wfc_c = wfc.astype(bf)
    wpj_c = wpj.astype(bf)

    shared = {
        "wq": wq.astype(bf), "wk": wk.astype(bf), "wv": wv.astype(bf),
        "bqk": bqk,
        "wap": np.asarray(w_attnproj, np.float32).astype(bf),
        "wfc": wfc_c,
        "bfc": bfc.astype(np.float32),
        "wpj": wpj_c,
    }
    in_maps = []
    for core in range(8):
        b, half = core // 2, core % 2
        xb = x[b]
        own = xb[half * TO:(half + 1) * TO]
        other = xb[(1 - half) * TO:(2 - half) * TO]
        m = dict(shared)
        m["x"] = np.ascontiguousarray(np.concatenate([own, other], 0))
        in_maps.append(m)
    return in_maps


def kernel(x, ln1_g, ln1_b, w_qkv, b_qkv, w_attnproj, b_attnproj,
           ln2_g, ln2_b, w_fc, b_fc, w_proj, b_proj):
    global LAST_RESULT
    in_maps = prepare_in_maps(
        x, ln1_g, ln1_b, w_qkv, b_qkv, w_attnproj, b_attnproj,
        ln2_g, ln2_b, w_fc, b_fc, w_proj, b_proj)

    if "nc" not in _CACHE:
        _CACHE["nc"] = _build()
    nc = _CACHE["nc"]

    LAST_RESULT = run_bass_kernel_spmd(nc, in_maps, core_ids=list(range(8)))

    out = np.empty((4, T, C), np.float32)
    for core in range(8):
        b, half = core // 2, core % 2
        out[b, half * TO:(half + 1) * TO] = LAST_RESULT.results[core]["out"]
    return out


# revision 13
# speedup vs baseline: 1.2027x; 1.1473x over previous
"""Trainium2 Bass kernel for a GPT-style transformer block.

B=4, T=2048, C=1024, H=16 heads (hd=64), D_FF=4096, fp32 I/O,
pre-LN, non-causal attention, tanh-approx GELU.

Sharding: 8 cores = 4 batch elements x 2 token-halves. Each core
computes attention K/V for its full batch element (dup of the K/V
projection for the other half -- avoids all collectives) and Q/MLP for
its own 1024 tokens. Host reorders tokens so each core's own tokens are
always rows 0..1023 -> identical NEFF on all 8 cores.

v2: bf16 activations/weights through attention (fast transposes, light
DMA), K/Q projection fused into the per-head-pair attention loop so exp
on the Activation engine overlaps PE work, stationary-reuse loop orders
(halved LDWEIGHTS), fp8e4m3 DoubleRow FFN (2 contraction rows per
partition), per-phase named scopes.
"""

import numpy as np
from contextlib import ExitStack

import concourse.bass as bass
import concourse.bacc as bacc
import concourse.mybir as mybir
from concourse import tile
from concourse.bass_utils import run_bass_kernel_spmd
from concourse.masks import make_identity

F32 = mybir.dt.float32
BF16 = mybir.dt.bfloat16
FP8 = mybir.dt.float8e4
AF = mybir.ActivationFunctionType
ALU = mybir.AluOpType
DR = mybir.MatmulPerfMode.DoubleRow

P = 128
T = 2048      # tokens per batch element (per core: kv tokens)
TO = 1024     # own tokens per core
C = 1024
H = 16
HD = 64
FF = 4096
NT = T // P   # 16 token tiles (kv)
NTO = TO // P  # 8 own token tiles
NC = C // P   # 8 channel tiles
NF = FF // P  # 32 ff tiles
EPS = 1e-5

USE_FP8_FFN = False
WFC_SCALE = 8.0
WPJ_SCALE = 16.0

_CACHE = {}
LAST_RESULT = None


def _ln_tile(nc, pools, src_ap, dstT, tslice, ident16, epsc):
    """LayerNorm one [128, C] token tile (gains folded into weights on
    host), cast to bf16, transpose into dstT[:, :, tslice] (dstT dtype)."""
    pool, spool, pps = pools
    st = spool.tile([P, 2, 6], F32, name="ln_st")
    for g in range(2):
        nc.vector.bn_stats(st[:, g], src_ap[:, g * 512:(g + 1) * 512])
    ag = spool.tile([P, 2], F32, name="ln_ag")
    nc.vector.bn_aggr(ag[:], st[:])
    std = spool.tile([P, 1], F32, name="ln_std")
    nc.scalar.activation(std[:], ag[:, 1:2], AF.Sqrt, bias=epsc)
    rinv = spool.tile([P, 1], F32, name="ln_rinv")
    nc.vector.reciprocal(rinv[:], std[:])
    xh = pool.tile([P, C], BF16, name="ln_xh")
    nc.vector.tensor_scalar(
        xh[:], src_ap, ag[:, 0:1], rinv[:], ALU.subtract, ALU.mult)
    for c in range(NC):
        tp = pps.tile([P, P], BF16, name="ln_tp")
        nc.tensor.transpose(tp[:], xh[:, c * P:(c + 1) * P], ident16)
        dst = dstT[:, c, tslice]
        if c % 2 == 0:
            nc.scalar.copy(dst, tp[:])
        else:
            nc.vector.tensor_copy(dst, tp[:])


def _build():
    nc = bacc.Bacc(None, target_bir_lowering=False)
    wdt = FP8 if USE_FP8_FFN else BF16

    # ---- DRAM I/O ----
    x_d = nc.dram_tensor("x", (T, C), F32, kind="ExternalInput")
    wq_d = nc.dram_tensor("wq", (C, C), BF16, kind="ExternalInput")
    wk_d = nc.dram_tensor("wk", (C, C), BF16, kind="ExternalInput")
    wv_d = nc.dram_tensor("wv", (C, C), BF16, kind="ExternalInput")
    bqk_d = nc.dram_tensor("bqk", (2 * C,), F32, kind="ExternalInput")
    wap_d = nc.dram_tensor("wap", (C, C), BF16, kind="ExternalInput")
    wfc_d = nc.dram_tensor("wfc", (C, FF), BF16, kind="ExternalInput")
    bfc_d = nc.dram_tensor("bfc", (FF,), F32, kind="ExternalInput")
    wpj_d = nc.dram_tensor("wpj", (FF, C), BF16, kind="ExternalInput")
    out_d = nc.dram_tensor("out", (TO, C), F32, kind="ExternalOutput")

    with tile.TileContext(nc) as tc, ExitStack() as top:
        cpool = top.enter_context(tc.tile_pool(name="const", bufs=1))
        epsc = cpool.tile([P, 1], F32, name="epsc")
        nc.vector.memset(epsc[:], EPS)
        ident16 = cpool.tile([P, P], BF16, name="ident16")
        make_identity(nc, ident16)
        ones_t = cpool.tile([P, HD], BF16, name="ones_t")
        nc.vector.memset(ones_t[:], 1.0)
        bqk_sb = cpool.tile([P, 2 * NC], F32, name="bqk_sb")
        nc.sync.dma_start(
            bqk_sb[:], bqk_d[:].rearrange("(j p) -> p j", p=P))
        bfc_sb = cpool.tile([P, NF], F32, name="bfc_sb")
        nc.sync.dma_start(
            bfc_sb[:], bfc_d[:].rearrange("(j p) -> p j", p=P))

        esA = top.enter_context(ExitStack())   # xhT (left)
        esBC = top.enter_context(ExitStack())  # vsb (right)
        esYW = top.enter_context(ExitStack())  # yT + wap: freed after D
        # yT lives from attention through phase D; allocate its pool below
        # const (before bigA) so left-side pool pops stay LIFO.
        yTp = esYW.enter_context(tc.tile_pool(name="yTp", bufs=1))

        # ============ Phase A: LN1 + transpose + V projection ============
        bigA = esA.enter_context(tc.tile_pool(name="bigA", bufs=1))
        xhT = bigA.tile([P, NC, T], BF16, name="xhT")  # 4 MB
        vsb = esBC.enter_context(
            tc.tile_pool(name="vsbp", bufs=1, side="right")).tile(
            [P, NT, H * (HD + 1)], BF16, name="vsb")
        with nc.named_scope("phA_ln_v"), ExitStack() as esAV:
            lnp = (esAV.enter_context(tc.tile_pool(name="ln_work", bufs=3)),
                   esAV.enter_context(tc.tile_pool(name="ln_stat", bufs=6)),
                   esAV.enter_context(
                       tc.tile_pool(name="ln_ps", bufs=2, space="PSUM")))
            xpool = esAV.enter_context(tc.tile_pool(name="xinp", bufs=3))
            wvp = esAV.enter_context(tc.tile_pool(name="wvp", bufs=1))
            psV = esAV.enter_context(
                tc.tile_pool(name="psV", bufs=4, space="PSUM"))
            wv_sb = wvp.tile([P, NC, C], BF16, name="wv_sb")
            nc.sync.dma_start(
                wv_sb[:], wv_d[:].rearrange("(c p) o -> p c o", p=P))
            for i in range(NT):
                xt = xpool.tile([P, C], F32, name="ln_x")
                nc.sync.dma_start(xt[:], x_d[i * P:(i + 1) * P, :])
                _ln_tile(nc, lnp, xt[:], xhT,
                         slice(i * P, (i + 1) * P), ident16, epsc[:])
                # V proj: stationary = token tile of xhT, reused across vc
                psv = {vc: psV.tile([P, 512], F32, name="psv")
                       for vc in range(2)}
                for c in range(NC):
                    for vc in range(2):
                        nc.tensor.matmul(
                            psv[vc][:], xhT[:, c, i * P:(i + 1) * P],
                            wv_sb[:, c, vc * 512:(vc + 1) * 512],
                            start=(c == 0), stop=(c == NC - 1))
                dstv = vsb[:, i].rearrange("p (h e) -> p h e", e=HD + 1)
                for vc in range(2):
                    nc.vector.tensor_copy(
                        dstv[:, vc * 8:(vc + 1) * 8, :HD],
                        psv[vc][:].rearrange("p (h d) -> p h d", d=HD))
                nc.gpsimd.memset(dstv[:, :, HD:], 1.0)

        # ========== Phase B+C: software-pipelined attention ==========
        # Act (exp) paces each head-pair j.  To keep the PE continuously
        # busy (and at full p-state), independent ready work -- K/Q
        # projection of j+1 and PV of j-1 -- is emitted in small units
        # BEFORE each Act-gated score pair, so the PE never head-of-line
        # blocks on a psS slot while useful work is pending.
        yT = yTp.tile([P, NC, TO], BF16, name="yT")
        with nc.named_scope("phBC_attn"), ExitStack() as esC:
            wkp = esC.enter_context(tc.tile_pool(name="wkp", bufs=2))
            wqp = esC.enter_context(tc.tile_pool(name="wqp", bufs=2))
            kTp = esC.enter_context(tc.tile_pool(name="kTp", bufs=2,
                                                 side="right"))
            qTp = esC.enter_context(tc.tile_pool(name="qTp", bufs=2,
                                                 side="right"))
            psKQ = esC.enter_context(
                tc.tile_pool(name="psKQ", bufs=2, space="PSUM"))
            psS = esC.enter_context(
                tc.tile_pool(name="psS", bufs=2, space="PSUM"))
            psO = esC.enter_context(
                tc.tile_pool(name="psO", bufs=2, space="PSUM"))
            ppool = esC.enter_context(tc.tile_pool(name="pT", bufs=42))
            scrp = esC.enter_context(tc.tile_pool(name="scrp", bufs=2))
            dinp = esC.enter_context(tc.tile_pool(name="dinp", bufs=2))
            wk_r = wk_d[:].rearrange("(c p) o -> p c o", p=P)
            wq_r = wq_d[:].rearrange("(c p) o -> p c o", p=P)
            kq = {}
            pTs = {}

            def kq_units(j):
                """6 closures computing kT/qT for head-pair j."""
                if j >= H // 2:
                    return []
                st = {}

                def setup(j=j):
                    wk_t = wkp.tile([P, NC, P], BF16, name="wk_t")
                    nc.sync.dma_start(
                        wk_t[:], wk_r[:, :, j * P:(j + 1) * P])
                    wq_t = wqp.tile([P, NC, P], BF16, name="wq_t")
                    nc.sync.dma_start(
                        wq_t[:], wq_r[:, :, j * P:(j + 1) * P])
                    st["w"] = (wk_t, wq_t)
                    kq[j] = (kTp.tile([P, T], BF16, name="kTj"),
                             qTp.tile([P, TO], BF16, name="qTj"))

                def khalf(tg, ch, j=j):
                    wk_t = st["w"][0]
                    kTj = kq[j][0]
                    if ch == 0:
                        st["pk", tg] = {
                            t2: psKQ.tile([P, 512], F32, name="pskq")
                            for t2 in range(2)}
                    pk = st["pk", tg]
                    for c in range(ch * 4, ch * 4 + 4):
                        for t2 in range(2):
                            toff = tg * 1024 + t2 * 512
                            nc.tensor.matmul(
                                pk[t2][:], wk_t[:, c],
                                xhT[:, c, toff:toff + 512],
                                start=(c == 0), stop=(c == NC - 1))
                    if ch == 1:
                        for t2 in range(2):
                            toff = tg * 1024 + t2 * 512
                            nc.vector.tensor_scalar_add(
                                kTj[:, toff:toff + 512], pk[t2][:],
                                bqk_sb[:, NC + j:NC + j + 1])

                def qhalf(ch, j=j):
                    wq_t = st["w"][1]
                    qTj = kq[j][1]
                    if ch == 0:
                        st["pq"] = {
                            t2: psKQ.tile([P, 512], F32, name="pskq")
                            for t2 in range(2)}
                    pq = st["pq"]
                    for c in range(ch * 4, ch * 4 + 4):
                        for t2 in range(2):
                            nc.tensor.matmul(
                                pq[t2][:], wq_t[:, c],
                                xhT[:, c, t2 * 512:(t2 + 1) * 512],
                                start=(c == 0), stop=(c == NC - 1))
                    if ch == 1:
                        for t2 in range(2):
                            nc.vector.tensor_scalar_add(
                                qTj[:, t2 * 512:(t2 + 1) * 512], pq[t2][:],
                                bqk_sb[:, j:j + 1])

                return [setup,
                        lambda: khalf(0, 0), lambda: khalf(0, 1),
                        lambda: khalf(1, 0), lambda: khalf(1, 1),
                        lambda: qhalf(0), lambda: qhalf(1)]

            def pv_units(j):
                """Per po: 4 chain units + 1 normalize unit (10 total)."""
                if j < 0:
                    return []
                units = []
                for po in (0, 64):
                    st = {}

                    def chain(kg, po=po, st=st, j=j):
                        h = 2 * j + (po // HD)
                        pT = pTs[j]
                        if kg == 0:
                            st["ops"] = {
                                qg: psO.tile([P, 512], F32, name="ops")
                                for qg in range(2)}
                        for k in range(kg * 4, kg * 4 + 4):
                            for qg in range(2):
                                nc.tensor.matmul(
                                    st["ops"][qg][:HD + 1, :],
                                    vsb[:, k,
                                        h * (HD + 1):(h + 1) * (HD + 1)],
                                    pT[po][k][:, qg * 512:(qg + 1) * 512],
                                    start=(k == 0), stop=(k == NT - 1))

                    def norm(po=po, st=st, j=j):
                        scrs = []
                        for qg in range(2):
                            scr = scrp.tile([P, 512], BF16, name="scr")
                            nc.vector.tensor_copy(
                                scr[HD:HD + 1, :],
                                st["ops"][qg][HD:HD + 1, :])
                            scrs.append(scr)
                        for qg in range(2):
                            dps = psKQ.tile([P, 512], F32, name="pskq")
                            nc.tensor.matmul(
                                dps[:HD, :], ones_t[HD:HD + 1, :],
                                scrs[qg][HD:HD + 1, :], start=True,
                                stop=True, tile_position=(HD, 0))
                            dinvT = dinp.tile([HD, 512], BF16, name="dinvT")
                            with nc.allow_low_precision(
                                    reason="bf16 1/denom for softmax"):
                                nc.vector.reciprocal(dinvT[:], dps[:HD, :])
                            nc.vector.tensor_tensor(
                                yT[po:po + HD, j,
                                   qg * 512:(qg + 1) * 512],
                                st["ops"][qg][:HD, :], dinvT[:], ALU.mult)

                    units += [lambda kg=kg, c=chain: c(kg)
                              for kg in range(4)]
                    units.append(norm)
                return units

            for u in kq_units(0):
                u()
            for j in range(H // 2 + 1):
                units = pv_units(j - 1) + kq_units(j + 1)
                if j < H // 2:
                    kTj, qTj = kq[j]
                    pT = {0: [None] * NT, 64: [None] * NT}
                    pTs[j] = pT
                    for k in range(NT):
                        if units:
                            units.pop(0)()
                        sps = {po: psS.tile([P, TO], F32, name="sps",
                                            tag="sps")
                               for po in (0, 64)}
                        for qc in range(2):
                            for po in (0, 64):
                                nc.tensor.matmul(
                                    sps[po][:, qc * 512:(qc + 1) * 512],
                                    kTj[po:po + HD, k * P:(k + 1) * P],
                                    qTj[po:po + HD,
                                        qc * 512:(qc + 1) * 512],
                                    start=True, stop=True)
                        for po in (0, 64):
                            pT[po][k] = ppool.tile([P, TO], BF16,
                                                   name="pT_t")
                            nc.scalar.activation(
                                pT[po][k][:], sps[po][:], AF.Exp,
                                scale=0.125)
                for u in units:
                    u()
                if j - 1 in pTs:
                    del pTs[j - 1]
        esA.close()   # free xhT
        esBC.close()  # free vsb

        # ========== Phase D: attn proj + residual + LN2 fused ==========
        x2 = top.enter_context(
            tc.tile_pool(name="x2p", bufs=1, side="right")).tile(
            [P, NTO, C], F32, name="x2")
        xh2T = top.enter_context(
            tc.tile_pool(name="bigE", bufs=1, side="right")).tile(
            [P, NC, TO], wdt, name="xh2T")
        wap_sb = esYW.enter_context(
            tc.tile_pool(name="wapp", bufs=1)).tile(
            [P, NC, C], BF16, name="wap_sb")
        nc.sync.dma_start(
            wap_sb[:], wap_d[:].rearrange("(c p) o -> p c o", p=P))
        # h2T (bf16, FFN1 out / FFN2 stationary); wfc streamed in halves
        h2T = top.enter_context(
            tc.tile_pool(name="h2Tp", bufs=1, side="right")).tile(
            [P, NF, TO], BF16, name="h2T")
        wfcp = top.enter_context(
            tc.tile_pool(name="wfcp", bufs=1, side="right"))
        wfc_r = wfc_d[:].rearrange("(c p) f -> p c f", p=P)
        wfc_t0 = wfcp.tile([P, NC, FF // 2], BF16, name="wfc_t")
        nc.sync.dma_start(wfc_t0[:], wfc_r[:, :, 0:FF // 2])
        with nc.named_scope("phD_proj_ln2"), ExitStack() as esD:
            xrp = esD.enter_context(tc.tile_pool(name="xrp", bufs=3))
            psD = esD.enter_context(
                tc.tile_pool(name="psD", bufs=4, space="PSUM"))
            ln2p = (esD.enter_context(tc.tile_pool(name="ln2_work", bufs=3)),
                    esD.enter_context(tc.tile_pool(name="ln2_stat", bufs=6)),
                    esD.enter_context(
                        tc.tile_pool(name="ln2_ps", bufs=2, space="PSUM")))
            for qt in range(NTO):
                xr = xrp.tile([P, C], F32, name="xr")
                nc.sync.dma_start(xr[:], x_d[qt * P:(qt + 1) * P, :])
                pd = {cc: psD.tile([P, 512], F32, name="psd")
                      for cc in range(2)}
                for c in range(NC):
                    for cc in range(2):
                        nc.tensor.matmul(
                            pd[cc][:], yT[:, c, qt * P:(qt + 1) * P],
                            wap_sb[:, c, cc * 512:(cc + 1) * 512],
                            start=(c == 0), stop=(c == NC - 1))
                for cc in range(2):
                    nc.vector.tensor_tensor(
                        x2[:, qt, cc * 512:(cc + 1) * 512], pd[cc][:],
                        xr[:, cc * 512:(cc + 1) * 512], ALU.add)
                _ln_tile(nc, ln2p, x2[:, qt], xh2T,
                         slice(qt * P, (qt + 1) * P), ident16, epsc[:])

        # ================= Phase F: FFN1 + gelu =================
        esYW.close()  # free yT, wap
        wpjp = top.enter_context(
            tc.tile_pool(name="wpjp", bufs=1, side="right"))
        wpj_r = wpj_d[:].rearrange("(f p) o -> p f o", p=P)
        wpj_t0 = wpjp.tile([P, NF, 512], BF16, name="wpj_t")
        nc.sync.dma_start(wpj_t0[:], wpj_r[:, :, 0:512])
        with nc.named_scope("phF_ffn1"), ExitStack() as esF:
            psF = esF.enter_context(
                tc.tile_pool(name="psF", bufs=4, space="PSUM"))
            for fh in range(2):
                if fh == 0:
                    wfc_t = wfc_t0
                else:
                    wfc_t = wfcp.tile([P, NC, FF // 2], BF16, name="wfc_t")
                    nc.sync.dma_start(
                        wfc_t[:], wfc_r[:, :, FF // 2:FF])
                for fj in range(NF // 2):
                    fjg = fh * (NF // 2) + fj
                    pf = {tch: psF.tile([P, 512], F32, name="psf")
                          for tch in range(2)}
                    for c in range(NC):
                        for tch in range(2):
                            nc.tensor.matmul(
                                pf[tch][:],
                                wfc_t[:, c, fj * P:(fj + 1) * P],
                                xh2T[:, c, tch * 512:(tch + 1) * 512],
                                start=(c == 0), stop=(c == NC - 1))
                    for tch in range(2):
                        nc.scalar.activation(
                            h2T[:, fjg, tch * 512:(tch + 1) * 512],
                            pf[tch][:], AF.Gelu_apprx_tanh,
                            bias=bfc_sb[:, fjg:fjg + 1])

        # ================= Phase G: FFN2 + residual + out =================
        with nc.named_scope("phG_ffn2"), ExitStack() as esG:
            psG = esG.enter_context(
                tc.tile_pool(name="psG", bufs=4, space="PSUM"))
            opool = esG.enter_context(tc.tile_pool(name="outp", bufs=3))
            for cc in range(2):
                if cc == 0:
                    wpj_t = wpj_t0
                else:
                    wpj_t = wpjp.tile([P, NF, 512], BF16, name="wpj_t")
                    nc.sync.dma_start(
                        wpj_t[:], wpj_r[:, :, cc * 512:(cc + 1) * 512])
                for qt in range(NTO):
                    pg = psG.tile([P, 512], F32, name="psg")
                    for f in range(NF):
                        nc.tensor.matmul(
                            pg[:], h2T[:, f, qt * P:(qt + 1) * P],
                            wpj_t[:, f, :],
                            start=(f == 0), stop=(f == NF - 1))
                    ot = opool.tile([P, 512], F32, name="ot")
                    nc.vector.tensor_tensor(
                        ot[:], pg[:],
                        x2[:, qt, cc * 512:(cc + 1) * 512], ALU.add)
                    nc.sync.dma_start(
                        out_d[qt * P:(qt + 1) * P, cc * 512:(cc + 1) * 512],
                        ot[:])

    nc.compile()
    return nc


def prepare_in_maps(x, ln1_g, ln1_b, w_qkv, b_qkv, w_attnproj, b_attnproj,
                    ln2_g, ln2_b, w_fc, b_fc, w_proj, b_proj):
    import ml_dtypes
    bf = ml_dtypes.bfloat16
    f8 = ml_dtypes.float8_e4m3

    x = np.asarray(x, np.float32)
    ln1_g = np.asarray(ln1_g, np.float32)
    ln1_b = np.asarray(ln1_b, np.float32)
    w_qkv = np.asarray(w_qkv, np.float32)
    b_qkv = np.asarray(b_qkv, np.float32)

    Wqkv = ln1_g[:, None] * w_qkv
    Bqkv = ln1_b @ w_qkv + b_qkv
    wq = np.ascontiguousarray(Wqkv[:, :C])
    wk = np.ascontiguousarray(Wqkv[:, C:2 * C])
    wv = np.ascontiguousarray(Wqkv[:, 2 * C:])
    bqk = np.concatenate([Bqkv[:C], Bqkv[C:2 * C]]).astype(np.float32)
    bv = Bqkv[2 * C:]
    assert np.all(bv == 0), "nonzero V bias not supported in this build"
    assert np.all(np.asarray(b_attnproj) == 0)
    assert np.all(np.asarray(b_proj) == 0)

    wfc = (np.asarray(ln2_g, np.float32)[:, None]
           * np.asarray(w_fc, np.float32))
    bfc = (np.asarray(ln2_b, np.float32) @ np.asarray(w_fc, np.float32)
           + np.asarray(b_fc, np.float32))
    wpj = np.asarray(w_proj, np.float32)

    wfc_c = wfc.astype(bf)
    wpj_c = wpj.astype(bf)

    shared = {
        "wq": wq.astype(bf), "wk": wk.astype(bf), "wv": wv.astype(bf),
        "bqk": bqk,
        "wap": np.asarray(w_attnproj, np.float32).astype(bf),
        "wfc": wfc_c,
        "bfc": bfc.astype(np.float32),
        "wpj": wpj_c,
    }
    in_maps = []
    for core in range(8):
        b, half = core // 2, core % 2
        xb = x[b]
        own = xb[half * TO:(half + 1) * TO]
        other = xb[(1 - half) * TO:(2 - half) * TO]
        m = dict(shared)
        m["x"] = np.ascontiguousarray(np.concatenate([own, other], 0))
        in_maps.append(m)
    return in_maps


def kernel(x, ln1_g, ln1_b, w_qkv, b_qkv, w_attnproj, b_attnproj,
           ln2_g, ln2_b, w_fc, b_fc, w_proj, b_proj):
    global LAST_RESULT
    in_maps = prepare_in_maps(
        x, ln1_g, ln1_b, w_qkv, b_qkv, w_attnproj, b_attnproj,
        ln2_g, ln2_b, w_fc, b_fc, w_proj, b_proj)

    if "nc" not in _CACHE:
        _CACHE["nc"] = _build()
    nc = _CACHE["nc"]

    LAST_RESULT = run_bass_kernel_spmd(nc, in_maps, core_ids=list(range(8)))

    out = np.empty((4, T, C), np.float32)
    for core in range(8):
        b, half = core // 2, core % 2
        out[b, half * TO:(half + 1) * TO] = LAST_RESULT.results[core]["out"]
    return out
